# revision 52
# baseline (speedup 1.0000x reference)
"""Trainium2 Bass kernel for nn_CausalSE: causal cumulative-average pooling
+ squeeze-excite gating, data-parallel over batch (one NeuronCore per batch
element).

Reference math per batch element (D=512, T=8192, chunk=16, Tc=512):
    avg    = cumsum(x, t) / (t+1)
    pooled = avg[:, 15::16]                          # [D, Tc]
    h      = relu(w1 @ pooled + b1)                  # [64, Tc]
    g      = sigmoid(w2 @ h + b2)                    # [D, Tc]
    out    = repeat(g, 16, t)[:, :T] * x

Kernel structure (x/out bf16 in HBM+SBUF, SE math fp32 in PSUM/SBUF,
causally pipelined over 2048-col t-blocks):

  * HBM layout is repacked on host to [128, 4, T] (partition-major) so each
    t-block moves as one or two large DMAs.
  * q = w1 @ chunk_sums(x) is computed WITHOUT materializing per-channel
    chunk sums: contract channels FIRST on the PE (y = w1 @ x over all
    columns, contiguous moving operand at full rate), then window-fold
    y [64, T] on the DVE — 8x less fold work than reducing x [512, T].
  * causal prefix via tensor_tensor_scan with carried initial (fp32), then
    h = relu(qs*scl + b1) in two DVE ops, g = sigmoid(w2 @ h + b2) on
    PE+ACT, gate-multiply in place in SBUF (DVE for 3 of 4 row tiles,
    GpSimd for the 4th), and 1 MB stores from ACT/GpSimd queues.
  * Emission is software-pipelined: block tb's q-matmul group precedes
    block tb-1's gate matmuls on the PE queue, so the PE never waits on
    the scan->h chain of the block it just produced.
"""

import sys

for _p in ("/opt/trn_rl_repo",):
    if _p not in sys.path:
        sys.path.insert(0, _p)

import numpy as np

B, D, T = 8, 512, 8192
DH = 64          # bottleneck dim = D // 8
CS = 16          # chunksize
TC = T // CS     # 512 chunks
NCORES = 8
NDT = D // 128   # 4 partition tiles of x / out
# t-block column spans for the causal pipeline: small first block so the
# PE starts ~5us earlier, small last block to shorten the serial tail.
_TBS = [512, 1536, 2048, 2048, 1536, 512]
TBLOCKS = [(sum(_TBS[:i]), tb) for i, tb in enumerate(_TBS)]

_compiled_nc = None


def build_nc():
    import concourse.tile as tile
    from concourse import bacc, mybir

    f32 = mybir.dt.float32
    bf16 = mybir.dt.bfloat16
    AF = mybir.ActivationFunctionType
    ALU = mybir.AluOpType
    AX = mybir.AxisListType

    # Bacc (not plain Bass): its finalize() runs the TRN2 sync-wait
    # legalization (move_matmul_waits_to_ldweights / event-semaphore
    # splitting) that walrus codegen requires.
    nc = bacc.Bacc("TRN2", target_bir_lowering=False)
    # x / out live in HBM as bf16, partition-major [128, NDT, T]: halves the
    # 32 MB/core fp32 HBM floor (2e-2 tolerance admits bf16 with ~3x margin)
    # and lets a whole t-block load in one DMA.
    x_d = nc.declare_dram_parameter("x", [128, NDT, T], bf16, isOutput=False)
    # weights pre-packed on host so each lands in ONE DMA (11 small serial
    # DMAs previously stalled the first q-matmuls until ~19us)
    w1t_d = nc.declare_dram_parameter("w1t", [128, NDT, DH], bf16, isOutput=False)
    w2t_d = nc.declare_dram_parameter("w2t", [DH, D], bf16, isOutput=False)
    b2_d = nc.declare_dram_parameter("b2", [128, NDT], f32, isOutput=False)
    bscl_d = nc.declare_dram_parameter("bscl", [DH, 1 + TC], f32, isOutput=False)
    out_d = nc.declare_dram_parameter("out", [128, NDT, T], bf16, isOutput=True)

    with tile.TileContext(nc) as tc:
        with (
            tc.tile_pool(name="xres", bufs=1) as xres,
            tc.tile_pool(name="small", bufs=1) as small,
            tc.tile_pool(name="psum_y", bufs=2, space="PSUM") as psum_y,
            tc.tile_pool(name="psum_g", bufs=4, space="PSUM") as psum_g,
        ):
            # x resident in SBUF: one [128, NDT, TB] bf16 tile per t-block
            xb = [
                xres.tile([128, NDT, TB], bf16, tag=f"x{tb}", name=f"x{tb}")
                for tb, (t0, TB) in enumerate(TBLOCKS)
            ]
            w1s = small.tile([128, NDT, DH], bf16, tag="w1")
            w2s = small.tile([DH, D], bf16, tag="w2")
            b2s = small.tile([128, NDT], f32, tag="b2")
            # bscl packs b1 (col 0) and the 1/(16(c+1)) scale row (cols 1:)
            bscl = small.tile([DH, 1 + TC], f32, tag="bscl")
            qsum = small.tile([DH, TC], f32, tag="qsum")  # chunk sums of y
            qs = small.tile([DH, TC], f32, tag="qs")    # causal raw prefix
            hf = small.tile([DH, TC], f32, tag="hf")    # qs * scl scratch
            hb = small.tile([DH, TC], bf16, tag="hb")   # relu(hf + b1)
            gs = [
                small.tile([128, TC], bf16, tag=f"g{di}", name=f"g{di}")
                for di in range(NDT)
            ]
            # full-resolution gate scratch for the 2x-mode DVE multiply
            gx = [
                small.tile([128, 2048], bf16, tag=f"gx{i}", name=f"gx{i}")
                for i in range(3)
            ]

            # Weights on the ACT queue (one DMA each, in first-use order);
            # x blocks stream in order on the sync queue so block 0's
            # completion fires first.
            nc.scalar.dma_start(bscl[:], bscl_d[:])
            nc.scalar.dma_start(w1s[:], w1t_d[:])
            nc.scalar.dma_start(w2s[:], w2t_d[:])
            nc.scalar.dma_start(b2s[:], b2_d[:])
            # dummy sigmoid: pulls the ~1.3us ACT table load off the first
            # block's critical path (runs while x still streams in)
            nc.gpsimd.memset(gs[0][:, 0:1], 0.0)
            nc.scalar.activation(
                gs[0][:, 0:1], gs[0][:, 0:1], AF.Sigmoid, bias=0.0
            )
            for tb, (t0, TB) in enumerate(TBLOCKS):
                nc.sync.dma_start(xb[tb][:], x_d[:, :, t0:t0 + TB])

            def gates(tb):
                """Gate matmuls + sigmoid + in-place multiply + stores for
                block tb (emitted one block behind the q pipeline)."""
                t0, TB = TBLOCKS[tb]
                CB = TB // CS
                c0 = t0 // CS
                xv4 = xb[tb].rearrange("p s (c j) -> p s c j", j=CS)
                for di in range(NDT):
                    gp = psum_g.tile([128, CB], f32, tag="g", name="gp")
                    nc.tensor.matmul(
                        gp[:],
                        w2s[:, di * 128:(di + 1) * 128],
                        hb[:, c0:c0 + CB],
                        start=True,
                        stop=True,
                    )
                    nc.scalar.activation(
                        gs[di][:, c0:c0 + CB], gp[:], AF.Sigmoid,
                        bias=b2s[:, di:di + 1],
                    )
                # gate-multiply in place. di 0-2: ACT (otherwise idle)
                # expands the gate to full resolution so the DVE multiply
                # has step-1 operands and runs in 2x packed mode (measured
                # 1.13us vs 2.2us per 2048-col tile). di 3 is split between
                # GpSimd (7/8, direct broadcast mult at ~33 G elem/s — it
                # has slack now that stores left its queue) and DVE (1/8).
                HB2 = 7 * CB // 8
                for di in range(NDT):
                    xv = xv4[:, di, :, :]
                    gv = (
                        gs[di][:, c0:c0 + CB]
                        .unsqueeze(2)
                        .broadcast_to([128, CB, CS])
                    )
                    if di < 3:
                        gxt = gx[di][:, 0:TB].rearrange(
                            "p (c j) -> p c j", j=CS
                        )
                        nc.scalar.activation(gxt, gv, AF.Copy)
                        nc.vector.tensor_tensor(
                            xb[tb][:, di, :], xb[tb][:, di, :],
                            gx[di][:, 0:TB], op=ALU.mult,
                        )
                    else:
                        nc.gpsimd.tensor_tensor(
                            xv[:, :HB2, :], xv[:, :HB2, :], gv[:, :HB2, :],
                            op=ALU.mult,
                        )
                        nc.vector.tensor_tensor(
                            xv[:, HB2:, :], xv[:, HB2:, :], gv[:, HB2:, :],
                            op=ALU.mult,
                        )
                # 1 MB stores, both on the sync queue (idle after the
                # loads; all loads are emitted ahead of any store so the
                # FIFO can't block them; the ACT queue would stall its
                # sigmoid/copy work, and SWDGE issue would cost GpSimd
                # ~0.8us/block of descriptor generation).
                nc.sync.dma_start(
                    out_d[:, 0:2, t0:t0 + TB], xb[tb][:, 0:2, :]
                )
                nc.sync.dma_start(
                    out_d[:, 2:4, t0:t0 + TB], xb[tb][:, 2:4, :]
                )

            # Causal pipeline over t-blocks.
            SUB = 512                       # y sub-chunk columns (1 PSUM bank)
            for tb, (t0, TB) in enumerate(TBLOCKS):
                CB = TB // CS
                c0 = t0 // CS
                # y = w1 @ x_block on the PE (contiguous moving, full rate;
                # each matmul output stays within one 512-col PSUM bank),
                # then fold y's 16-col windows into chunk sums on the DVE
                # (8x less fold work than reducing x directly). Folding
                # 1024 cols per reduce halves the DVE dispatch overhead.
                pos = 0
                while pos < TB:
                    seg = min(2 * SUB, TB - pos)
                    yp = psum_y.tile([DH, 2 * SUB], f32, tag="y", name="yp")
                    for h in range(seg // SUB):
                        for ki in range(NDT):
                            nc.tensor.matmul(
                                yp[:, h * SUB:(h + 1) * SUB],
                                w1s[:, ki, :],
                                xb[tb][
                                    :, ki, pos + h * SUB:pos + (h + 1) * SUB
                                ],
                                start=(ki == 0),
                                stop=(ki == NDT - 1),
                            )
                    cs0 = c0 + pos // CS
                    nc.vector.reduce_sum(
                        qsum[:, cs0:cs0 + seg // CS],
                        yp[:, :seg].rearrange("p (c j) -> p c j", j=CS),
                        axis=AX.X,
                    )
                    pos += seg
                # running causal prefix over this block (carry = last col)
                # (data1 is ignored under op1=bypass; qsum avoids a
                # dependency on the bscl weight DMA)
                nc.vector.tensor_tensor_scan(
                    qs[:, c0:c0 + CB],
                    qsum[:, c0:c0 + CB],
                    qsum[:, c0:c0 + CB],
                    0.0 if tb == 0 else qs[:, c0 - 1:c0],
                    op0=ALU.add,
                    op1=ALU.bypass,
                )
                # h = relu(qs * scl + b1), downcast to bf16 for the PE
                nc.vector.tensor_tensor(
                    hf[:, c0:c0 + CB], qs[:, c0:c0 + CB],
                    bscl[:, 1 + c0:1 + c0 + CB], op=ALU.mult,
                )
                nc.vector.tensor_scalar(
                    hb[:, c0:c0 + CB], hf[:, c0:c0 + CB], bscl[:, 0:1], 0.0,
                    op0=ALU.add, op1=ALU.max,
                )
                # software-pipeline skew: block tb's gates are emitted after
                # block tb+1's q-matmuls so the PE never waits on the
                # scan->h chain — except block 0, whose gates go out
                # immediately (the PE would otherwise just idle waiting for
                # block 1's load).
                if tb == 0:
                    gates(0)
                elif tb >= 2:
                    gates(tb - 1)
            gates(len(TBLOCKS) - 1)
    # run_bass_via_pjrt serializes nc.m as-is; Bacc defers register
    # allocation and TRN2 sync-wait legalization to finalize(), so it must
    # run here or walrus rejects the BIR.
    nc.finalize()
    return nc


def _host_inputs(x, w1, b1, w2, b2, chunksize):
    import ml_dtypes

    bf = ml_dtypes.bfloat16
    x = np.asarray(x, dtype=np.float32).astype(bf)
    # repack [B, 512, T] -> [B, 128, 4, T]: partition p holds rows
    # {p, 128+p, 256+p, 384+p}
    x = np.ascontiguousarray(x.reshape(B, NDT, 128, T).transpose(0, 2, 1, 3))
    w1 = np.asarray(w1, dtype=np.float32)
    b1 = np.ascontiguousarray(np.asarray(b1, dtype=np.float32))
    w2 = np.asarray(w2, dtype=np.float32)
    b2 = np.ascontiguousarray(np.asarray(b2, dtype=np.float32))
    cs = int(chunksize)
    assert cs == CS and x.shape == (B, 128, NDT, T), (cs, x.shape)
    # w1t packed [128, NDT, DH]: row p, slot ki = w1.T row 128*ki + p
    w1t = np.ascontiguousarray(
        w1.T.astype(bf).reshape(NDT, 128, DH).transpose(1, 0, 2)
    )
    w2t = np.ascontiguousarray(w2.T.astype(bf))           # [DH, D]
    b2p = np.ascontiguousarray(b2.reshape(NDT, 128).T)    # [128, NDT]
    scale = np.broadcast_to(
        1.0 / (CS * np.arange(1, TC + 1, dtype=np.float32)), (DH, TC)
    )
    bscl = np.ascontiguousarray(
        np.concatenate([np.broadcast_to(b1[:, None], (DH, 1)), scale], axis=1)
    )
    shared = dict(w1t=w1t, w2t=w2t, b2=b2p, bscl=bscl)
    return x, shared


def _unpack_out(out):
    # [128, 4, T] bf16 -> [512, T] fp32
    return (
        np.asarray(out).transpose(1, 0, 2).reshape(D, T).astype(np.float32)
    )


def kernel(x, w1, b1, w2, b2, chunksize):
    global _compiled_nc
    from concourse.bass_utils import run_bass_kernel_spmd

    x, shared = _host_inputs(x, w1, b1, w2, b2, chunksize)
    if _compiled_nc is None:
        _compiled_nc = build_nc()
    in_maps = [
        {"x": np.ascontiguousarray(x[i]), **shared} for i in range(NCORES)
    ]
    res = run_bass_kernel_spmd(_compiled_nc, in_maps, list(range(NCORES)))
    out = np.stack(
        [_unpack_out(res.results[i]["out"]) for i in range(NCORES)], axis=0
    )
    return out
